# revision 7
# baseline (speedup 1.0000x reference)
"""Trainium2 Bass kernel for MockFP8Linear: out = x @ (W * block_scale)^T.

Strategy: data-parallel over tokens across 8 NeuronCores (no collectives).
Each core:
  - dequantizes + transposes the full weight on-chip: load f32 row-tiles,
    cast to bf16 on ACT, PE-transpose 128x128 blocks, fold the per-block
    dequant scale into the PSUM->SBUF eviction (DVE tensor_scalar_mul).
    W^T (bf16, 8 MB) stays resident in SBUF for the whole kernel.
  - streams 128-token tiles of its x shard: load f32, cast bf16,
    PE-transpose to put the contraction dim on partitions, then runs
    lhsT(=x^T tile, stationary) @ rhs(=W^T slice, moving) accumulating
    fp32 in PSUM over the 16 k-blocks; evicts to SBUF and DMAs out.
"""

import os
import sys

import numpy as np

for _p in ("/opt/trn_rl_repo", "/root/.axon_site/_ro/trn_rl_repo"):
    if os.path.isdir(_p) and _p not in sys.path:
        sys.path.append(_p)

TOKENS, IN_F, OUT_F = 16384, 2048, 2048
NCORES = 8
TSH = TOKENS // NCORES  # tokens per core
P = 128
KB = IN_F // P  # contraction blocks
TB = TSH // P  # token tiles per core
OBL = OUT_F // P  # out_features blocks (scale granularity)
NCH = OUT_F // 512  # psum chunks of the output row-tile

_cached = None


def _build():
    from contextlib import ExitStack

    import concourse.tile as tile
    from concourse import bacc, mybir
    from concourse.masks import make_identity

    f32 = mybir.dt.float32
    bf16 = mybir.dt.bfloat16

    nc = bacc.Bacc("TRN2", target_bir_lowering=False, debug=False, num_devices=NCORES)
    x_d = nc.dram_tensor("x", [TSH, IN_F], f32, kind="ExternalInput").ap()
    w_d = nc.dram_tensor("w", [OUT_F, IN_F], f32, kind="ExternalInput").ap()
    s_d = nc.dram_tensor("s", [P, OBL, KB], f32, kind="ExternalInput").ap()
    o_d = nc.dram_tensor("out", [TSH, OUT_F], f32, kind="ExternalOutput").ap()

    with tile.TileContext(nc) as tc:
        with ExitStack() as ctx:
            const = ctx.enter_context(tc.tile_pool(name="const", bufs=1))
            ident = const.tile([P, P], bf16)
            make_identity(nc, ident)
            scales = const.tile([P, OBL, KB], f32)
            nc.sync.dma_start(scales[:], s_d[:])

            wT_pool = ctx.enter_context(tc.tile_pool(name="wT", bufs=1))
            wT = wT_pool.tile([P, KB * OUT_F], bf16)  # [i_p, ib*OUT_F + o]

            wnat_pool = ctx.enter_context(tc.tile_pool(name="wnat", bufs=2))
            wbf_pool = ctx.enter_context(tc.tile_pool(name="wbf", bufs=2))
            xnat_pool = ctx.enter_context(tc.tile_pool(name="xnat", bufs=2))
            xbf_pool = ctx.enter_context(tc.tile_pool(name="xbf", bufs=2))
            xT_pool = ctx.enter_context(tc.tile_pool(name="xT", bufs=2))
            outsb_pool = ctx.enter_context(tc.tile_pool(name="outsb", bufs=2))
            tps_pool = ctx.enter_context(tc.tile_pool(name="tps", bufs=2, space="PSUM"))
            ops_pool = ctx.enter_context(tc.tile_pool(name="ops", bufs=6, space="PSUM"))

            # ---- W dequant + transpose phase ----
            for ob in range(OBL):
                wnat = wnat_pool.tile([P, IN_F], f32, tag="wnat")
                nc.sync.dma_start(wnat[:], w_d[bass_ds(ob * P, P), :])
                wbf = wbf_pool.tile([P, IN_F], bf16, tag="wbf")
                nc.scalar.copy(wbf[:], wnat[:])
                for ib0 in range(0, KB, 4):
                    ps = tps_pool.tile([P, 512], bf16, tag="tps")
                    for j in range(4):
                        ib = ib0 + j
                        nc.tensor.transpose(
                            ps[:, j * P : (j + 1) * P],
                            wbf[:, ib * P : (ib + 1) * P],
                            ident[:],
                        )
                    for j in range(4):
                        ib = ib0 + j
                        nc.vector.tensor_scalar_mul(
                            wT[:, ib * OUT_F + ob * P : ib * OUT_F + (ob + 1) * P],
                            ps[:, j * P : (j + 1) * P],
                            scales[:, ob, ib : ib + 1],
                        )

            # ---- main loop over token tiles ----
            for tt in range(TB):
                xnat = xnat_pool.tile([P, IN_F], f32, tag="xnat")
                nc.sync.dma_start(xnat[:], x_d[bass_ds(tt * P, P), :])
                xbf = xbf_pool.tile([P, IN_F], bf16, tag="xbf")
                nc.scalar.copy(xbf[:], xnat[:])
                xT = xT_pool.tile([P, KB * P], bf16, tag="xT")  # [i_p, ib*P + t]
                for ib0 in range(0, KB, 4):
                    ps = tps_pool.tile([P, 512], bf16, tag="tps")
                    for j in range(4):
                        ib = ib0 + j
                        nc.tensor.transpose(
                            ps[:, j * P : (j + 1) * P],
                            xbf[:, ib * P : (ib + 1) * P],
                            ident[:],
                        )
                    nc.scalar.copy(xT[:, ib0 * P : (ib0 + 4) * P], ps[:])

                psum = [
                    ops_pool.tile([P, 512], f32, tag="ops", name=f"ops_{tt}_{nb}")
                    for nb in range(NCH)
                ]
                for ib in range(KB):
                    for nb in range(NCH):
                        nc.tensor.matmul(
                            psum[nb][:],
                            lhsT=xT[:, ib * P : (ib + 1) * P],
                            rhs=wT[:, ib * OUT_F + nb * 512 : ib * OUT_F + (nb + 1) * 512],
                            start=(ib == 0),
                            stop=(ib == KB - 1),
                        )

                outsb = outsb_pool.tile([P, OUT_F], f32, tag="outsb")
                for nb in range(NCH):
                    if nb % 2 == 0:
                        nc.scalar.copy(outsb[:, nb * 512 : (nb + 1) * 512], psum[nb][:])
                    else:
                        nc.vector.tensor_copy(
                            outsb[:, nb * 512 : (nb + 1) * 512], psum[nb][:]
                        )
                nc.sync.dma_start(o_d[bass_ds(tt * P, P), :], outsb[:])

    nc.compile()
    return nc


def bass_ds(start, size):
    from concourse.bass import ds

    return ds(start, size)


def _get_compiled():
    global _cached
    if _cached is None:
        _cached = _build()
    return _cached


def _ensure_ntff_hook():
    """Register the axon NTFF profile hook (boot skips it when
    antenv.axon_hooks is absent from the image). Only needed for trace=True."""
    import sys as _sys
    import types as _types

    if "antenv.axon_hooks" not in _sys.modules:
        import antenv

        mod = _types.ModuleType("antenv.axon_hooks")
        mod._hook = None

        def set_axon_ntff_profile_hook(h):
            mod._hook = h

        def get_axon_ntff_profile_hook():
            return mod._hook

        mod.set_axon_ntff_profile_hook = set_axon_ntff_profile_hook
        mod.get_axon_ntff_profile_hook = get_axon_ntff_profile_hook
        _sys.modules["antenv.axon_hooks"] = mod
        antenv.axon_hooks = mod
    mod = _sys.modules["antenv.axon_hooks"]
    if mod._hook is None:
        from trn_agent_boot.trn_boot import _ntff_profile_via_ctypes

        hook = _ntff_profile_via_ctypes("/opt/axon/libaxon_pjrt.so")
        if hook is not None:
            mod.set_axon_ntff_profile_hook(hook)


def run(x, weight, weight_scale, trace=False, trace_cores=None):
    from concourse.bass_utils import run_bass_kernel_spmd

    nc = _get_compiled()

    x = np.ascontiguousarray(np.asarray(x, dtype=np.float32))
    weight = np.ascontiguousarray(np.asarray(weight, dtype=np.float32))
    weight_scale = np.asarray(weight_scale, dtype=np.float32)
    scales_b = np.ascontiguousarray(
        np.broadcast_to(weight_scale[None, :, :], (P, OBL, KB)).astype(np.float32)
    )

    in_maps = [
        {
            "x": np.ascontiguousarray(x[c * TSH : (c + 1) * TSH]),
            "w": weight,
            "s": scales_b,
        }
        for c in range(NCORES)
    ]
    kwargs = {}
    if trace:
        try:
            _ensure_ntff_hook()
        except Exception as e:  # tracing is best-effort; the run still works
            print(f"ntff hook registration failed ({e}); tracing may be skipped")
        kwargs = dict(trace=True, trace_cores=trace_cores or [0])
    res = run_bass_kernel_spmd(nc, in_maps, core_ids=list(range(NCORES)), **kwargs)
    out = np.concatenate([res.results[c]["out"] for c in range(NCORES)], axis=0)
    return out, res


def kernel(x, weight, weight_scale):
    out, _ = run(x, weight, weight_scale)
    return out
